# revision 26
# baseline (speedup 1.0000x reference)
"""Multi-head attention on 8 TRN2 NeuronCores — v2.

Problem: queries [B,N,L,H,E], keys [B,N,S,H,E], values [B,N,S,H,D]
         out[b,n,l,h,:] = softmax(Q[b,n,l,h,:] @ K[b,n,:,h,:]^T / sqrt(E)) @ V[b,n,:,h,:]
with B,N,L,S,H,E,D = 4,7,512,512,8,64,64.

Sharding: head-parallel — core c computes all B*N=28 (b,n) slices for head h=c.

v2 design (per slice, L=S=512, E=D=64, P=128):
  1. QK row-tiled 2x concurrent: Q^T duplicated in both partition halves;
     K chunks c0/c1 in rows 0-63, c2/c3 in rows 64-127. Two matmul "units"
     each issue a (0,0)- and a (64,0)-row-group MM that run concurrently in
     the PE array -> 4 score chunks [128s, 512l] in ~2x512 column-cycles.
  2. exp split: ScalarE ACTIVATE exact exp on chunks 0-2 (fp16 out, one
     [128,1536] instruction); DVE does chunk 3 via a Schraudolph bit-trick:
     i16 = rne(score*A + Bc), reinterpret bits as fp16 ~= exp(score/8).
     (GPSIMD cannot access PSUM, so it cannot help with exp of scores.)
  3. PV with attn chunks as stationary: po[128l, 65] per l-chunk accumulates
     lhsT=attnT[sc, lc*128:+128], rhs=[V_sc | ones] (FD=65). Column 64 is
     the softmax denominator. Output partition dim = l -> no transpose and
     no partition-broadcast epilogue.
  4. DVE copies po [128, 260] PSUM->SBUF as fp16 (DMA cannot read PSUM;
     fp16 halves output HBM traffic); raw numerator+denominator DMA'd out;
     host does the divide. num/den cancels the fp16 scale error.

PSUM budget (8 banks): scoresA [128,1536]x2 (6) + scoresB [128,512]x1 (1)
 + po [128,260]x1 (1) = 8.
"""

import numpy as np

B, N, L, S, H, E, D = 4, 7, 512, 512, 8, 64, 64
NS = B * N          # 28 slices per core
NP = NS // 2        # 14 slice-pairs (DMA granularity)
P = 128
SC = S // P         # 4 s-chunks
LC = L // P         # 4 l-chunks
SCALE = 1.0 / float(np.sqrt(E))

# per-slice input pack [128, 1028] fp16:
#   [0:512)    Q2: rows 0-63 = Q^T [E, L], rows 64-127 = same Q^T (copy)
#   [512:768)  K2: rows 0-63 = [K_c0 | K_c1] ([E,128] each), rows 64-127 = [K_c2 | K_c3]
#   [768:1028) VO: 4 chunks [128, 65] = [V_sc (64 cols) | ones]
QOFF, KOFF, VOFF = 0, 512, 768
SLC = 1028          # cols per slice
# Schraudolph: i16 = rne(x*EA + EB); bits as fp16 ~= exp(x/8)
EA = 1024.0 * float(np.log2(np.e)) / 8.0
EB = 15360.0 - 60.0

_CACHE = {}


def _build_program():
    import concourse.mybir as mybir
    import concourse.tile as tile
    from concourse import bacc
    import concourse.bass as bass

    f32 = mybir.dt.float32
    f16 = mybir.dt.float16
    i16 = mybir.dt.int16
    Exp = mybir.ActivationFunctionType.Exp
    MUL = mybir.AluOpType.mult
    ADD = mybir.AluOpType.add

    nc = bacc.Bacc("TRN2", target_bir_lowering=False, debug=False)
    inp = nc.dram_tensor("inp", [NP, P, 2 * SLC], f16, kind="ExternalInput").ap()
    o = nc.dram_tensor("o", [NS, P, 260], f16, kind="ExternalOutput").ap()

    with tile.TileContext(nc) as tc:
        with (
            tc.tile_pool(name="inpool", bufs=1) as in_pool,
            tc.tile_pool(name="attnA", bufs=1) as aA_pool,
            tc.tile_pool(name="attnB", bufs=1) as aB_pool,
            tc.tile_pool(name="osb", bufs=1) as osb_pool,
            tc.tile_pool(name="psA", bufs=1, space=bass.MemorySpace.PSUM) as psA_pool,
            tc.tile_pool(name="psB", bufs=1, space=bass.MemorySpace.PSUM) as psB_pool,
            tc.tile_pool(name="po", bufs=1, space=bass.MemorySpace.PSUM) as po_pool,
        ):
            # --- warm-up ---------------------------------------------------
            # HAM: back-to-back dummy matmuls give the PE a continuous busy
            # window so the clock gate opens (1.2 -> 2.4 GHz) before the
            # steady-state pipeline begins.
            warm = in_pool.tile([P, L], f16, tag="warm")
            # memset on GpSimd: its prologue finishes ~2us before DVE's, so
            # the warm-up matmuls (which gate on this tile) start earlier.
            nc.gpsimd.memset(warm[:], 1.0)
            # Preload ScalarE exp table (~2.7us once) with a tiny ACTIVATE.
            dummyA = osb_pool.tile([1, 8], f32, tag="dumA")
            nc.scalar.activation(dummyA[:], warm[0:1, 0:8], Exp, scale=SCALE)
            # Preload DVE/GpSimd tensor_scalar paths (Q7 wrapper etc.).
            dummyB = aB_pool.tile([1, 8], i16, tag="dumB")
            nc.vector.tensor_scalar(dummyB[:], warm[0:1, 0:8], EA, EB, MUL, ADD)
            wps = psB_pool.tile([P, L], f32, tag="b0")
            # ~12 cold matmuls = two full HAM windows of continuous PE busy,
            # so the clock gate opens (1.2 -> 2.4 GHz) regardless of window
            # phase; fewer leaves HAM oscillating k=4/8 all run (measured).
            for _ in range(6):
                nc.tensor.matmul(
                    wps[:], lhsT=warm[:, 0:P], rhs=warm[:], start=True, stop=True
                )

            # --- input DMA, 3 pairs ahead ---------------------------------
            in_tiles = {}

            def load_pair(p, split=False):
                if p < NP and p not in in_tiles:
                    t = in_pool.tile([P, 2 * SLC], f16, tag=f"t{p % 4}", name=f"in{p}")
                    if split:
                        # first pair: land slice 0's half first so QK(0)
                        # starts ~0.8us earlier
                        nc.sync.dma_start(t[:, 0:SLC], inp[p][:, 0:SLC])
                        nc.sync.dma_start(t[:, SLC:2 * SLC], inp[p][:, SLC:2 * SLC])
                    else:
                        nc.sync.dma_start(t[:], inp[p])
                    in_tiles[p] = t

            load_pair(0, split=True)
            for p in range(1, 3):
                load_pair(p)

            def emit_qk(k, in_t, j):
                """Score chunks for slice k. j = slice's half of the pair tile."""
                q2 = in_t[:, j * SLC + QOFF: j * SLC + QOFF + L]
                k2 = in_t[:, j * SLC + KOFF: j * SLC + KOFF + 2 * P]
                psA = psA_pool.tile([P, 3 * L], f32, tag=f"A{k % 2}")
                psB = psB_pool.tile([P, L], f32, tag="b0")
                # unit 0: chunk0 (rows 0-63) || chunk2 (rows 64-127)
                nc.tensor.matmul(psA[:, 0:L], lhsT=k2[0:E, 0:P], rhs=q2[0:E, :],
                                 start=True, stop=True)
                nc.tensor.matmul(psA[:, 2 * L:3 * L], lhsT=k2[E:P, 0:P], rhs=q2[E:P, :],
                                 start=True, stop=True)
                # unit 1: chunk1 || chunk3
                nc.tensor.matmul(psA[:, L:2 * L], lhsT=k2[0:E, P:2 * P], rhs=q2[0:E, :],
                                 start=True, stop=True)
                nc.tensor.matmul(psB[:], lhsT=k2[E:P, P:2 * P], rhs=q2[E:P, :],
                                 start=True, stop=True)
                return psA, psB

            def emit_exp(k, psA, psB, split=False):
                aA = aA_pool.tile([P, 3 * L], f16, tag=f"A{k % 2}")
                if split:
                    # drain: two ACTs so the last PV's sc0/sc1 matmuls can
                    # start while c2's exp still runs
                    nc.scalar.activation(aA[:, 0:2 * L], psA[:, 0:2 * L], Exp, scale=SCALE)
                    nc.scalar.activation(aA[:, 2 * L:3 * L], psA[:, 2 * L:3 * L], Exp, scale=SCALE)
                else:
                    nc.scalar.activation(aA[:], psA[:], Exp, scale=SCALE)
                aB = aB_pool.tile([P, L], i16, tag=f"B{k % 2}")
                nc.vector.tensor_scalar(aB[:], psB[:], EA, EB, MUL, ADD)
                return aA, aB

            def emit_pv(k, in_t, j, aA, aB, sc_major=False):
                f16aB = aB[:].bitcast(f16)
                vo = in_t[:, j * SLC + VOFF: j * SLC + VOFF + SC * 65]
                po = po_pool.tile([P, LC * 65], f32, tag="po0")
                order = [(lc, sc) for sc in range(SC) for lc in range(LC)] \
                    if sc_major else [(lc, sc) for lc in range(LC) for sc in range(SC)]
                for lc, sc in order:
                        if sc < 2:
                            st = aA[:, sc * L + lc * P: sc * L + (lc + 1) * P]
                        elif sc == 2:
                            st = aA[:, 2 * L + lc * P: 2 * L + (lc + 1) * P]
                        else:
                            st = f16aB[:, lc * P:(lc + 1) * P]
                        nc.tensor.matmul(
                            po[:, lc * 65:(lc + 1) * 65],
                            lhsT=st,
                            rhs=vo[:, sc * 65:(sc + 1) * 65],
                            start=(sc == 0),
                            stop=(sc == SC - 1),
                            skip_group_check=sc_major,
                        )
                return po

            osb_tiles = {}

            def emit_tail(kk, it, jj, at, last=False):
                po = emit_pv(kk, it, jj, *at)
                osb = osb_pool.tile([P, 260], f16, tag=f"o{kk % 3}", name=f"osb{kk}")
                nc.vector.tensor_copy(osb[:], po[:])
                nc.sync.dma_start(o[kk], osb[:])

            pend = []
            for k in range(NS):
                pair, j = k // 2, k % 2
                in_t = in_tiles[pair]
                if j == 1:
                    load_pair(pair + 3)
                psA, psB = emit_qk(k, in_t, j)
                attn = emit_exp(k, psA, psB)
                pend.append((k, in_t, j, attn))
                if len(pend) > 1:
                    pend_item = pend.pop(0)
                    emit_tail(*pend_item)
            while pend:
                pend_item = pend.pop(0)
                emit_tail(*pend_item, last=True)
    nc.compile()
    return nc


def _prep_inputs(queries, keys, values):
    """Pack per-core fp16 inputs. Core c gets head h=c."""
    q = np.asarray(queries, dtype=np.float32)
    k = np.asarray(keys, dtype=np.float32)
    v = np.asarray(values, dtype=np.float32)

    # Q^T / K^T per slice: [H, NS, E, L]
    qt = np.ascontiguousarray(q.transpose(3, 0, 1, 4, 2)).reshape(H, NS, E, L)
    kt = np.ascontiguousarray(k.transpose(3, 0, 1, 4, 2)).reshape(H, NS, E, S)
    q2 = np.concatenate([qt, qt], axis=2)                     # [H, NS, 128, 512]
    ktc = kt.reshape(H, NS, E, SC, P)
    k2 = np.concatenate(                                       # [H, NS, 128, 256]
        [
            ktc[:, :, :, 0:2].reshape(H, NS, E, 2 * P),
            ktc[:, :, :, 2:4].reshape(H, NS, E, 2 * P),
        ],
        axis=2,
    )
    # VO: [H, NS, sc, s, 65] = [V | ones] -> [H, NS, 128, 260]
    vt = v.transpose(3, 0, 1, 2, 4).reshape(H, NS, SC, P, D)
    vo = np.ones((H, NS, SC, P, 65), dtype=np.float32)
    vo[..., 0:D] = vt
    vo = np.ascontiguousarray(vo.transpose(0, 1, 3, 2, 4)).reshape(H, NS, P, SC * 65)

    inp = np.concatenate([q2, k2, vo], axis=3).astype(np.float16)  # [H, NS, 128, 1028]
    inp = np.ascontiguousarray(
        inp.reshape(H, NP, 2, P, SLC).transpose(0, 1, 3, 2, 4)
    ).reshape(H, NP, P, 2 * SLC)
    return [{"inp": inp[c]} for c in range(H)]


def _run(in_maps, trace=False, tmpdir=None):
    from concourse.bass_utils import run_bass_kernel_spmd

    if "nc" not in _CACHE:
        _CACHE["nc"] = _build_program()
    kwargs = {}
    if tmpdir is not None:
        kwargs["tmpdir"] = tmpdir
    return run_bass_kernel_spmd(
        _CACHE["nc"], in_maps, core_ids=list(range(H)), trace=trace, **kwargs
    )


def kernel(queries, keys, values, _trace=False, _results_out=None, _tmpdir=None):
    in_maps = _prep_inputs(queries, keys, values)
    res = _run(in_maps, trace=_trace, tmpdir=_tmpdir)
    if _results_out is not None:
        _results_out.append(res)
    # res.results[c]["o"]: [NS, 128, 260]
    raw = np.stack([res.results[c]["o"] for c in range(H)], axis=0).astype(np.float32)
    raw = raw.reshape(H, NS, P, LC, 65)
    num = raw[..., 0:D]            # [H, NS, p, lc, D]
    den = raw[..., D:D + 1]
    out = num / den                # [H, NS, p, lc, D]
    # l = lc*128 + p -> axes (NS, lc, p, H, D) then merge (lc, p) -> L
    out = out.transpose(1, 3, 2, 0, 4).reshape(B, N, L, H, D)
    return np.ascontiguousarray(out.astype(np.float32))


# revision 27
# speedup vs baseline: 1.1768x; 1.1768x over previous
"""Multi-head attention on 8 TRN2 NeuronCores — v2.

Problem: queries [B,N,L,H,E], keys [B,N,S,H,E], values [B,N,S,H,D]
         out[b,n,l,h,:] = softmax(Q[b,n,l,h,:] @ K[b,n,:,h,:]^T / sqrt(E)) @ V[b,n,:,h,:]
with B,N,L,S,H,E,D = 4,7,512,512,8,64,64.

Sharding: head-parallel — core c computes all B*N=28 (b,n) slices for head h=c.

v2 design (per slice, L=S=512, E=D=64, P=128):
  1. QK row-tiled 2x concurrent: Q^T duplicated in both partition halves;
     K chunks c0/c1 in rows 0-63, c2/c3 in rows 64-127. Two matmul "units"
     each issue a (0,0)- and a (64,0)-row-group MM that run concurrently in
     the PE array -> 4 score chunks [128s, 512l] in ~2x512 column-cycles.
  2. exp split: ScalarE ACTIVATE exact exp on chunks 0-2 (fp16 out, one
     [128,1536] instruction); DVE does chunk 3 via a Schraudolph bit-trick:
     i16 = rne(score*A + Bc), reinterpret bits as fp16 ~= exp(score/8).
     (GPSIMD cannot access PSUM, so it cannot help with exp of scores.)
  3. PV with attn chunks as stationary: po[128l, 65] per l-chunk accumulates
     lhsT=attnT[sc, lc*128:+128], rhs=[V_sc | ones] (FD=65). Column 64 is
     the softmax denominator. Output partition dim = l -> no transpose and
     no partition-broadcast epilogue.
  4. DVE copies po [128, 260] PSUM->SBUF as fp16 (DMA cannot read PSUM;
     fp16 halves output HBM traffic); raw numerator+denominator DMA'd out;
     host does the divide. num/den cancels the fp16 scale error.

PSUM budget (8 banks): scoresA [128,1536]x2 (6) + scoresB [128,512]x1 (1)
 + po [128,260]x1 (1) = 8.
"""

import numpy as np

B, N, L, S, H, E, D = 4, 7, 512, 512, 8, 64, 64
NS = B * N          # 28 slices per core
NP = NS // 2        # 14 slice-pairs (DMA granularity)
P = 128
SC = S // P         # 4 s-chunks
LC = L // P         # 4 l-chunks
SCALE = 1.0 / float(np.sqrt(E))

# per-slice input pack [128, 1028] fp16:
#   [0:512)    Q2: rows 0-63 = Q^T [E, L], rows 64-127 = same Q^T (copy)
#   [512:768)  K2: rows 0-63 = [K_c0 | K_c1] ([E,128] each), rows 64-127 = [K_c2 | K_c3]
#   [768:1028) VO: 4 chunks [128, 65] = [V_sc (64 cols) | ones]
QOFF, KOFF, VOFF = 0, 512, 768
SLC = 1028          # cols per slice
# Schraudolph: i16 = rne(x*EA + EB); bits as fp16 ~= exp(x/8)
EA = 1024.0 * float(np.log2(np.e)) / 8.0
EB = 15360.0 - 60.0

_CACHE = {}


def _build_program():
    import concourse.mybir as mybir
    import concourse.tile as tile
    from concourse import bacc
    import concourse.bass as bass

    f32 = mybir.dt.float32
    f16 = mybir.dt.float16
    i16 = mybir.dt.int16
    Exp = mybir.ActivationFunctionType.Exp
    MUL = mybir.AluOpType.mult
    ADD = mybir.AluOpType.add

    nc = bacc.Bacc("TRN2", target_bir_lowering=False, debug=False)
    inp = nc.dram_tensor("inp", [NP, P, 2 * SLC], f16, kind="ExternalInput").ap()
    o = nc.dram_tensor("o", [NP, P, 2 * 260], f16, kind="ExternalOutput").ap()

    with tile.TileContext(nc) as tc:
        with (
            tc.tile_pool(name="inpool", bufs=1) as in_pool,
            tc.tile_pool(name="attnA", bufs=1) as aA_pool,
            tc.tile_pool(name="attnB", bufs=1) as aB_pool,
            tc.tile_pool(name="osb", bufs=1) as osb_pool,
            tc.tile_pool(name="psA", bufs=1, space=bass.MemorySpace.PSUM) as psA_pool,
            tc.tile_pool(name="psB", bufs=1, space=bass.MemorySpace.PSUM) as psB_pool,
            tc.tile_pool(name="po", bufs=1, space=bass.MemorySpace.PSUM) as po_pool,
        ):
            # --- warm-up ---------------------------------------------------
            # HAM: back-to-back dummy matmuls give the PE a continuous busy
            # window so the clock gate opens (1.2 -> 2.4 GHz) before the
            # steady-state pipeline begins.
            warm = in_pool.tile([P, L], f16, tag="warm")
            # memset on GpSimd: its prologue finishes ~2us before DVE's, so
            # the warm-up matmuls (which gate on this tile) start earlier.
            nc.gpsimd.memset(warm[:], 1.0)
            # Preload ScalarE exp table (~2.7us once) with a tiny ACTIVATE.
            dummyA = osb_pool.tile([1, 8], f32, tag="dumA")
            nc.scalar.activation(dummyA[:], warm[0:1, 0:8], Exp, scale=SCALE)
            # Preload DVE/GpSimd tensor_scalar paths (Q7 wrapper etc.).
            dummyB = aB_pool.tile([1, 8], i16, tag="dumB")
            nc.vector.tensor_scalar(dummyB[:], warm[0:1, 0:8], EA, EB, MUL, ADD)
            wps = psB_pool.tile([P, L], f32, tag="b0")
            # ~12 cold matmuls = two full HAM windows of continuous PE busy,
            # so the clock gate opens (1.2 -> 2.4 GHz) regardless of window
            # phase; fewer leaves HAM oscillating k=4/8 all run (measured).
            for _ in range(6):
                nc.tensor.matmul(
                    wps[:], lhsT=warm[:, 0:P], rhs=warm[:], start=True, stop=True
                )

            # --- input DMA, 3 pairs ahead ---------------------------------
            in_tiles = {}

            def load_pair(p, split=False):
                if p < NP and p not in in_tiles:
                    t = in_pool.tile([P, 2 * SLC], f16, tag=f"t{p % 4}", name=f"in{p}")
                    if split:
                        # first pair: land slice 0's half first so QK(0)
                        # starts ~0.8us earlier
                        nc.sync.dma_start(t[:, 0:SLC], inp[p][:, 0:SLC])
                        nc.sync.dma_start(t[:, SLC:2 * SLC], inp[p][:, SLC:2 * SLC])
                    else:
                        nc.sync.dma_start(t[:], inp[p])
                    in_tiles[p] = t

            load_pair(0, split=True)
            for p in range(1, 3):
                load_pair(p)

            def emit_qk(k, in_t, j):
                """Score chunks for slice k. j = slice's half of the pair tile."""
                q2 = in_t[:, j * SLC + QOFF: j * SLC + QOFF + L]
                k2 = in_t[:, j * SLC + KOFF: j * SLC + KOFF + 2 * P]
                psA = psA_pool.tile([P, 3 * L], f32, tag=f"A{k % 2}")
                psB = psB_pool.tile([P, L], f32, tag="b0")
                # unit 0: chunk0 (rows 0-63) || chunk2 (rows 64-127)
                nc.tensor.matmul(psA[:, 0:L], lhsT=k2[0:E, 0:P], rhs=q2[0:E, :],
                                 start=True, stop=True)
                nc.tensor.matmul(psA[:, 2 * L:3 * L], lhsT=k2[E:P, 0:P], rhs=q2[E:P, :],
                                 start=True, stop=True)
                # unit 1: chunk1 || chunk3
                nc.tensor.matmul(psA[:, L:2 * L], lhsT=k2[0:E, P:2 * P], rhs=q2[0:E, :],
                                 start=True, stop=True)
                nc.tensor.matmul(psB[:], lhsT=k2[E:P, P:2 * P], rhs=q2[E:P, :],
                                 start=True, stop=True)
                return psA, psB

            def emit_exp(k, psA, psB, split=False):
                aA = aA_pool.tile([P, 3 * L], f16, tag=f"A{k % 2}")
                if split:
                    # drain: two ACTs so the last PV's sc0/sc1 matmuls can
                    # start while c2's exp still runs
                    nc.scalar.activation(aA[:, 0:2 * L], psA[:, 0:2 * L], Exp, scale=SCALE)
                    nc.scalar.activation(aA[:, 2 * L:3 * L], psA[:, 2 * L:3 * L], Exp, scale=SCALE)
                else:
                    nc.scalar.activation(aA[:], psA[:], Exp, scale=SCALE)
                aB = aB_pool.tile([P, L], i16, tag=f"B{k % 2}")
                nc.vector.tensor_scalar(aB[:], psB[:], EA, EB, MUL, ADD)
                return aA, aB

            def emit_pv(k, in_t, j, aA, aB, sc_major=False):
                f16aB = aB[:].bitcast(f16)
                vo = in_t[:, j * SLC + VOFF: j * SLC + VOFF + SC * 65]
                po = po_pool.tile([P, LC * 65], f32, tag="po0")
                order = [(lc, sc) for sc in range(SC) for lc in range(LC)] \
                    if sc_major else [(lc, sc) for lc in range(LC) for sc in range(SC)]
                for lc, sc in order:
                        if sc < 2:
                            st = aA[:, sc * L + lc * P: sc * L + (lc + 1) * P]
                        elif sc == 2:
                            st = aA[:, 2 * L + lc * P: 2 * L + (lc + 1) * P]
                        else:
                            st = f16aB[:, lc * P:(lc + 1) * P]
                        nc.tensor.matmul(
                            po[:, lc * 65:(lc + 1) * 65],
                            lhsT=st,
                            rhs=vo[:, sc * 65:(sc + 1) * 65],
                            start=(sc == 0),
                            stop=(sc == SC - 1),
                            skip_group_check=sc_major,
                        )
                return po

            osb_tiles = {}

            def emit_tail(kk, it, jj, at, last=False):
                po = emit_pv(kk, it, jj, *at)
                pp = kk // 2
                if pp not in osb_tiles:
                    osb_tiles[pp] = osb_pool.tile([P, 2 * 260], f16, tag=f"o{pp % 3}", name=f"osb{pp}")
                osb = osb_tiles[pp]
                nc.vector.tensor_copy(osb[:, (kk % 2) * 260:(kk % 2 + 1) * 260], po[:])
                if kk % 2 == 1:
                    nc.sync.dma_start(o[pp], osb[:])
                    del osb_tiles[pp]

            pend = []
            for k in range(NS):
                pair, j = k // 2, k % 2
                in_t = in_tiles[pair]
                if j == 1:
                    load_pair(pair + 3)
                psA, psB = emit_qk(k, in_t, j)
                attn = emit_exp(k, psA, psB)
                pend.append((k, in_t, j, attn))
                if len(pend) > 1:
                    pend_item = pend.pop(0)
                    emit_tail(*pend_item)
            while pend:
                pend_item = pend.pop(0)
                emit_tail(*pend_item, last=True)
    nc.compile()
    return nc


def _prep_inputs(queries, keys, values):
    """Pack per-core fp16 inputs. Core c gets head h=c."""
    q = np.asarray(queries, dtype=np.float32)
    k = np.asarray(keys, dtype=np.float32)
    v = np.asarray(values, dtype=np.float32)

    # Q^T / K^T per slice: [H, NS, E, L]
    qt = np.ascontiguousarray(q.transpose(3, 0, 1, 4, 2)).reshape(H, NS, E, L)
    kt = np.ascontiguousarray(k.transpose(3, 0, 1, 4, 2)).reshape(H, NS, E, S)
    q2 = np.concatenate([qt, qt], axis=2)                     # [H, NS, 128, 512]
    ktc = kt.reshape(H, NS, E, SC, P)
    k2 = np.concatenate(                                       # [H, NS, 128, 256]
        [
            ktc[:, :, :, 0:2].reshape(H, NS, E, 2 * P),
            ktc[:, :, :, 2:4].reshape(H, NS, E, 2 * P),
        ],
        axis=2,
    )
    # VO: [H, NS, sc, s, 65] = [V | ones] -> [H, NS, 128, 260]
    vt = v.transpose(3, 0, 1, 2, 4).reshape(H, NS, SC, P, D)
    vo = np.ones((H, NS, SC, P, 65), dtype=np.float32)
    vo[..., 0:D] = vt
    vo = np.ascontiguousarray(vo.transpose(0, 1, 3, 2, 4)).reshape(H, NS, P, SC * 65)

    inp = np.concatenate([q2, k2, vo], axis=3).astype(np.float16)  # [H, NS, 128, 1028]
    inp = np.ascontiguousarray(
        inp.reshape(H, NP, 2, P, SLC).transpose(0, 1, 3, 2, 4)
    ).reshape(H, NP, P, 2 * SLC)
    return [{"inp": inp[c]} for c in range(H)]


def _run(in_maps, trace=False, tmpdir=None):
    from concourse.bass_utils import run_bass_kernel_spmd

    if "nc" not in _CACHE:
        _CACHE["nc"] = _build_program()
    kwargs = {}
    if tmpdir is not None:
        kwargs["tmpdir"] = tmpdir
    return run_bass_kernel_spmd(
        _CACHE["nc"], in_maps, core_ids=list(range(H)), trace=trace, **kwargs
    )


def kernel(queries, keys, values, _trace=False, _results_out=None, _tmpdir=None):
    in_maps = _prep_inputs(queries, keys, values)
    res = _run(in_maps, trace=_trace, tmpdir=_tmpdir)
    if _results_out is not None:
        _results_out.append(res)
    # res.results[c]["o"]: [NP, 128, 520] -> [NS, 128, 260]
    raw = np.stack([res.results[c]["o"] for c in range(H)], axis=0).astype(np.float32)
    raw = raw.reshape(H, NP, P, 2, 260).transpose(0, 1, 3, 2, 4).reshape(H, NS, P, LC, 65)
    num = raw[..., 0:D]            # [H, NS, p, lc, D]
    den = raw[..., D:D + 1]
    out = num / den                # [H, NS, p, lc, D]
    # l = lc*128 + p -> axes (NS, lc, p, H, D) then merge (lc, p) -> L
    out = out.transpose(1, 3, 2, 0, 4).reshape(B, N, L, H, D)
    return np.ascontiguousarray(out.astype(np.float32))
